# revision 17
# baseline (speedup 1.0000x reference)
"""Trainium2 Bass kernel for nn_DiffHead_38757784879894.

Data-parallel over 8 NeuronCores: each core processes 32 of the 256
(b*nw) windows through the full network. All BN(eval) affine transforms
are folded into the adjacent linear ops on the host; matmuls run in
bf16 with fp32 PSUM accumulation; elementwise math stays fp32.

Self-contained: hardcodes all shapes; only needs numpy + the concourse
stack available in the environment.
"""
import math
from contextlib import ExitStack

import numpy as np
import ml_dtypes

import concourse.tile as tile
import concourse.bass_isa as bass_isa
from concourse import mybir, bacc
from concourse.bass_utils import run_bass_kernel_spmd

AF = mybir.ActivationFunctionType
OP = mybir.AluOpType
F32 = mybir.dt.float32
BF16 = mybir.dt.bfloat16

DIM, NL, NB, NW, NF, BTOT = 256, 3, 2, 8, 100, 256
NCORES = 8
WPC = BTOT // NCORES          # windows per core
P = 128
NCH = DIM // P                # 2 channel chunks
BNI = 1.0 / math.sqrt(1.0 + 1e-5)
L3 = NL * NF                  # 300

BF16_NP = ml_dtypes.bfloat16


# ----------------------------------------------------------------------------
# Host-side weight folding + packing
# ----------------------------------------------------------------------------

def _pack_pw(W):
    """(256,256) fp: out lhsT [kp, ic, oc, o] -> (128, 2*2*128)."""
    a = W.T.reshape(NCH, P, NCH, P)          # [ic, kp, oc, o]
    return np.transpose(a, (1, 0, 2, 3)).reshape(P, NCH * NCH * P)


def _pack_bias(b):
    """(256,) -> (128, 2) [p, chunk]."""
    return b.reshape(NCH, P).T.copy()


def prep_weights(params):
    """Fold BN/eval scales + mean-normalizers into weights; return named arrays."""
    p = {k: np.asarray(v, np.float64) for k, v in params.items()}
    out = {}

    # t_conv depthwise taps (3 layers): tw [p, c, ly, k], tbh = 0.5*bias
    tw = p['t_conv_w'].reshape(NL, DIM, 3)
    out['tw'] = ('f32', np.transpose(tw.reshape(NL, NCH, P, 3), (2, 1, 0, 3)).reshape(P, NCH * NL * 3))
    tb = p['t_conv_b'].reshape(NL, NCH, P)
    out['tbh'] = ('f32', (0.5 * np.transpose(tb, (2, 1, 0))).reshape(P, NCH * NL))

    # DiffMixer
    s_pre = p['dm_pre_g'] * BNI
    b_pre = p['dm_pre_b']
    s0 = p['dm_d0_g'] * BNI
    W0s = s0[:, None] * p['dm_d0_w']
    W0 = W0s * s_pre[None, :]
    b0 = s0 * p['dm_d0_b'] + p['dm_d0_beta'] + W0s @ b_pre
    s1 = p['dm_d1_g'] * BNI
    W1 = (s1[:, None] * p['dm_d1_w']) * s_pre[None, :]
    b1 = s1 * p['dm_d1_b'] + p['dm_d1_beta']
    out['dm0w'] = ('bf16', _pack_pw(W0))
    out['dm0b'] = ('f32', _pack_bias(b0))
    out['dm1w'] = ('bf16', _pack_pw(W1))
    out['dm1b'] = ('f32', _pack_bias(b1))

    # diff encoder blocks
    for i in range(NB):
        dt_ = p['de_dw_w'][i].reshape(DIM, 3)
        out[f'dtw{i}'] = ('f32', np.transpose(dt_.reshape(NCH, P, 3), (1, 0, 2)).reshape(P, NCH * 3))
        s = p['de_g'][i] * BNI
        Wp = s[:, None] * p['de_pw_w'][i]
        bp = s * (p['de_pw_w'][i] @ p['de_dw_b'][i] + p['de_pw_b'][i]) + p['de_beta'][i]
        out[f'de{i}w'] = ('bf16', _pack_pw(Wp))
        out[f'de{i}b'] = ('f32', _pack_bias(bp))

    # ResNet stem conv1: lhsT rows r = m*9 + ky*3 + kx, cols o (64)
    Wc1 = p['rn_c1_w'] * (p['rn_c1_g'] * BNI)[:, None, None, None]   # (64, 6, 3, 3)
    out['c1w'] = ('bf16', np.transpose(Wc1, (1, 2, 3, 0)).reshape(54, 64))
    out['c1b'] = ('f32', p['rn_c1_b'].reshape(64, 1))

    # conv2 tap-paired: 4 paired lhsT of K=128 + 1 single K=64.
    # pair i<3: rows[0:64]=tap(ky=i,kx=0), rows[64:128]=tap(ky=i,kx=2)
    # pair 3:   rows[0:64]=tap(ky=0,kx=1), rows[64:128]=tap(ky=2,kx=1)
    # single:   tap(ky=1,kx=1)
    Wc2 = p['rn_c2_w'] * (p['rn_c2_g'] * BNI)[:, None, None, None]   # (256, 64, 3, 3)
    a = np.transpose(Wc2.reshape(NCH, P, 64, 3, 3), (2, 3, 4, 0, 1))  # [ic, ky, kx, oc, o]
    pairs = np.zeros((P, 4, NCH, P))
    for ky in range(3):
        pairs[0:64, ky] = a[:, ky, 0]
        pairs[64:P, ky] = a[:, ky, 2]
    pairs[0:64, 3] = a[:, 0, 1]
    pairs[64:P, 3] = a[:, 2, 1]
    out['c2wp'] = ('bf16', pairs.reshape(P, 4 * NCH * P))
    out['c2ws'] = ('bf16', a[:, 1, 1].reshape(64, NCH * P))
    out['c2b'] = ('f32', _pack_bias(p['rn_c2_b']))

    # basic block convs: lhsT [icp, icc, t, oc, o]
    for nm, wkey, gkey, bkey in (('b1', 'rn_b1_w', 'rn_b1_g', 'rn_b1_b'),
                                 ('b2', 'rn_b2_w', 'rn_b2_g', 'rn_b2_b')):
        Wb = p[wkey] * (p[gkey] * BNI)[:, None, None, None]           # (256,256,3,3)
        a = Wb.reshape(NCH, P, NCH, P, 3, 3)                          # [oc,o,icc,icp,ky,kx]
        out[f'{nm}w'] = ('bf16', np.transpose(a, (3, 2, 4, 5, 0, 1)).reshape(P, NCH * 9 * NCH * P))
        out[f'{nm}b'] = ('f32', _pack_bias(p[bkey]))

    # SE: attn1 <- d1dsum/300 via se1_w1 ; attn2 <- hsum/625 via se2_w1
    for nm, w1k, w2k, scale in (('se1', 'se1_w1', 'se1_w2', 1.0 / 300.0),
                                ('se2', 'se2_w1', 'se2_w2', 1.0 / 625.0)):
        Wa = p[w1k] * scale                                           # (64, 256)
        a = Wa.T.reshape(NCH, P, 64)                                  # [kc, kp, o]
        out[f'{nm}a'] = ('bf16', np.transpose(a, (1, 0, 2)).reshape(P, NCH * 64))
        Wb_ = p[w2k]                                                  # (256, 64)
        a = Wb_.T.reshape(64, NCH, P)                                 # [kp, oc, o]
        out[f'{nm}b'] = ('bf16', a.reshape(64, NCH * P))

    # fuse: hcat = [attn1*hsum (scale 1/625) ; attn2*d1dsum (scale 1/300)]
    Wf = p['fuse_w'].copy()                                           # (256, 512)
    Wf[:, :DIM] *= 1.0 / 625.0
    Wf[:, DIM:] *= 1.0 / 300.0
    a = Wf.T.reshape(4, P, NCH, P)                                    # [kc, kp, oc, o]
    out['fusew'] = ('bf16', np.transpose(a, (1, 0, 2, 3)).reshape(P, 4 * NCH * P))
    out['fuseb'] = ('f32', _pack_bias(p['fuse_b']))

    out['ones'] = ('bf16', np.ones((P, 1)))
    return out


def pack_weights(wdict):
    """Pack named arrays into one (128, N) array per dtype group.

    Returns (wb_arr bf16, wf_arr f32, offsets: name -> (group, col0, rows, cols)).
    """
    offsets = {}
    cols = {'bf16': 0, 'f32': 0}
    for name, (grp, arr) in wdict.items():
        r, c = arr.shape
        offsets[name] = (grp, cols[grp], r, c)
        cols[grp] += c
    wb = np.zeros((P, cols['bf16']), dtype=BF16_NP)
    wf = np.zeros((P, cols['f32']), dtype=np.float32)
    for name, (grp, arr) in wdict.items():
        _, c0, r, c = offsets[name]
        if grp == 'bf16':
            wb[:r, c0:c0 + c] = arr.astype(BF16_NP)
        else:
            wf[:r, c0:c0 + c] = arr.astype(np.float32)
    return wb, wf, offsets


# ----------------------------------------------------------------------------
# Device program
# ----------------------------------------------------------------------------

class _Views:
    """AP views into the packed weight tiles."""

    def __init__(self, nc, wbt, wft, offsets):
        def view(name, shape):
            grp, c0, r, c = offsets[name]
            t = wbt if grp == 'bf16' else wft
            ap = t[0:r, c0:c0 + c]
            if len(shape) > 1:
                dims = ' '.join(f'a{i}' for i in range(len(shape)))
                kw = {f'a{i}': s for i, s in enumerate(shape)}
                ap = ap.rearrange(f'p ({dims}) -> p {dims}', **kw)
            return ap

        self.tw = view('tw', (NCH, NL, 3))
        self.tbh = view('tbh', (NCH, NL))
        self.dm0w = view('dm0w', (NCH, NCH, P))
        self.dm0b = view('dm0b', (NCH,))
        self.dm1w = view('dm1w', (NCH, NCH, P))
        self.dm1b = view('dm1b', (NCH,))
        self.dtw = [view(f'dtw{i}', (NCH, 3)) for i in range(NB)]
        self.dew = [view(f'de{i}w', (NCH, NCH, P)) for i in range(NB)]
        self.deb = [view(f'de{i}b', (NCH,)) for i in range(NB)]
        self.c1w = view('c1w', (64,))
        self.c1b = view('c1b', (1,))
        self.c2wp = view('c2wp', (4, NCH, P))
        self.c2ws = view('c2ws', (NCH, P))
        self.c2b = view('c2b', (NCH,))
        self.b1w = view('b1w', (NCH, 9, NCH, P))
        self.b1b = view('b1b', (NCH,))
        self.b2w = view('b2w', (NCH, 9, NCH, P))
        self.b2b = view('b2b', (NCH,))
        self.se1a = view('se1a', (NCH, 64))
        self.se1b = view('se1b', (NCH, P))
        self.se2a = view('se2a', (NCH, 64))
        self.se2b = view('se2b', (NCH, P))
        self.fusew = view('fusew', (4, NCH, P))
        self.fuseb = view('fuseb', (NCH,))
        self.ones = view('ones', (1,))


def build_nc(nwin=WPC, repeat=1, debug=(), offsets=None):
    """Build + compile the per-core program. `debug` is a set of stage names
    to dump as extra DRAM outputs (per-window slices along columns)."""
    nc = bacc.Bacc("TRN2", target_bir_lowering=False, debug=False, num_devices=1)
    nbf = max(c0 + c for g, c0, r, c in offsets.values() if g == 'bf16')
    nf32 = max(c0 + c for g, c0, r, c in offsets.values() if g == 'f32')
    xin = nc.dram_tensor("xin", [P, NCH * nwin * L3], F32, kind="ExternalInput")
    wbd = nc.dram_tensor("wb", [P, nbf], BF16, kind="ExternalInput")
    wfd = nc.dram_tensor("wf", [P, nf32], F32, kind="ExternalInput")
    outd = nc.dram_tensor("out", [DIM, nwin], F32, kind="ExternalOutput")

    dbg_t = {}

    def dbg(name, ap, w, rows, cols):
        if name not in debug:
            return
        if name not in dbg_t:
            dbg_t[name] = nc.dram_tensor(
                f"dbg_{name}", [rows, nwin * cols],
                ap.dtype, kind="ExternalOutput")
        nc.sync.dma_start(dbg_t[name].ap()[0:rows, w * cols:(w + 1) * cols], ap)

    with tile.TileContext(nc) as tc, ExitStack() as ctx:
        wpool = ctx.enter_context(tc.tile_pool(name="w", bufs=1))
        persist = ctx.enter_context(tc.tile_pool(name="persist", bufs=1))
        wa = ctx.enter_context(tc.tile_pool(name="wa", bufs=3))
        pacc = ctx.enter_context(tc.tile_pool(name="pacc", bufs=10))
        psum = ctx.enter_context(tc.tile_pool(name="psum", bufs=8, space="PSUM"))

        wbt = wpool.tile([P, nbf], BF16)
        nc.sync.dma_start(wbt[:], wbd.ap()[:])
        wft = wpool.tile([P, nf32], F32)
        nc.sync.dma_start(wft[:], wfd.ap()[:])
        V = _Views(nc, wbt, wft, offsets)

        xin_v = xin.ap().rearrange("p (c w l) -> p c w l", c=NCH, w=nwin)

        # persistent per-window-parity buffers
        hsum = persist.tile([P, NCH, nwin], F32)
        d1sum = persist.tile([P, NCH, nwin], F32)
        simsb, imcol, h1p, h2p, rp = ([], [], [], [], [])
        for b in range(3):
            simsb.append([
                persist.tile([P, 3, P], BF16, name=f"simsb{b}_0x", tag=f"simsb{b}_0x"),
                persist.tile([P, 3, P], BF16, name=f"simsb{b}_1x", tag=f"simsb{b}_1x")])
            imcol.append(persist.tile([54, 2500], BF16, name=f"imcolx{b}", tag=f"imcolx{b}"))
        for b in range(2):
            # sim_sb[b][half]: (128, 3, 128) per m-triple; per-m 128 cols:
            # [0]=0, [1:51]=odd l, [51:64]=pad, [64:114]=even l, [114:128]=pad
            # h1d: [0:64]=h1 padded, [64:128]=same shifted +2 cols (for kx=0|2 tap pairs)
            # h1e: [0:64]=h1 padded, [64:128]=same shifted +2 rows (for ky=0|2, kx=1 pair)
            h1p.append((persist.tile([P, 52 * 52], BF16, name=f"h1d{b}", tag=f"h1d{b}"),
                        persist.tile([P, 52 * 52], BF16, name=f"h1e{b}", tag=f"h1e{b}")))
            h2p.append(persist.tile([P, NCH, 27 * 27], BF16, name=f"h2p{b}", tag=f"h2p{b}"))
            rp.append(persist.tile([P, NCH, 27 * 27], BF16, name=f"rp{b}", tag=f"rp{b}"))
            nc.gpsimd.memset(h1p[b][0][:], 0.0)
            nc.gpsimd.memset(h1p[b][1][:], 0.0)
            nc.gpsimd.memset(h2p[b][:], 0.0)
            nc.gpsimd.memset(rp[b][:], 0.0)

        for _rep in range(repeat):
            for w in range(nwin):
                b = w % 2
                b3 = w % 3
                # ---- A: load window
                x_t = wa.tile([P, NCH, L3], F32, tag="x_t")
                nc.sync.dma_start(x_t[:], xin_v[:, :, w, :])

                # ---- B: t_conv (depthwise k3 pad1) + relu + 0.5*(t+x)
                y = wa.tile([P, NCH, L3], F32, tag="y")
                th = wa.tile([P, NCH, L3], F32, tag="th")
                for c in range(NCH):
                    for ly in range(NL):
                        xs = x_t[:, c, ly * NF:(ly + 1) * NF]
                        nc.vector.tensor_scalar(
                            y[:, c, ly * NF:(ly + 1) * NF], xs,
                            V.tw[:, c, ly, 1:2], None, OP.mult)
                        nc.vector.scalar_tensor_tensor(
                            y[:, c, ly * NF + 1:(ly + 1) * NF],
                            x_t[:, c, ly * NF:(ly + 1) * NF - 1],
                            V.tw[:, c, ly, 0:1],
                            y[:, c, ly * NF + 1:(ly + 1) * NF], OP.mult, OP.add)
                        nc.vector.scalar_tensor_tensor(
                            y[:, c, ly * NF:(ly + 1) * NF - 1],
                            x_t[:, c, ly * NF + 1:(ly + 1) * NF],
                            V.tw[:, c, ly, 2:3],
                            y[:, c, ly * NF:(ly + 1) * NF - 1], OP.mult, OP.add)
                        nc.scalar.activation(
                            th[:, c, ly * NF:(ly + 1) * NF],
                            y[:, c, ly * NF:(ly + 1) * NF],
                            AF.Relu, bias=V.tbh[:, c, ly:ly + 1], scale=0.5)
                ol = wa.tile([P, NCH, L3], F32, tag="ol")
                for c in range(NCH):
                    nc.vector.scalar_tensor_tensor(
                        ol[:, c, :], x_t[:, c, :], 0.5, th[:, c, :], OP.mult, OP.add)
                dbg('ol', ol[:].rearrange("p a b -> p (a b)"), w, P, NCH * L3)

                # ---- C: DiffMixer
                olb = wa.tile([P, NCH, L3], BF16, tag="olb")
                for c in range(NCH):
                    nc.scalar.activation(olb[:, c, :], ol[:, c, :], AF.Copy)
                xd = wa.tile([P, NCH, L3], BF16, tag="xd")
                for c in range(NCH):
                    ov = ol[:, c, :].rearrange("p (ly l) -> p ly l", ly=NL)
                    xv = xd[:, c, :].rearrange("p (ly l) -> p ly l", ly=NL)
                    nc.vector.tensor_tensor(xv[:, :, 1:NF], ov[:, :, 1:NF], ov[:, :, 0:NF - 1], OP.subtract)
                    nc.vector.tensor_tensor(xv[:, :, 0:1], ov[:, :, 1:2], ov[:, :, 0:1], OP.subtract)
                d0 = wa.tile([P, NCH, L3], F32, tag="d0")
                d1 = wa.tile([P, NCH, L3], F32, tag="d1")
                for wv, bv, rhs, dst in ((V.dm0w, V.dm0b, olb, d0), (V.dm1w, V.dm1b, xd, d1)):
                    for oc in range(NCH):
                        ps = psum.tile([P, L3], F32, tag="pp")
                        for ic in range(NCH):
                            nc.tensor.matmul(ps[:], wv[:, ic, oc, :], rhs[:, ic, :],
                                             start=(ic == 0), stop=(ic == NCH - 1))
                        nc.scalar.activation(dst[:, oc, :], ps[:], AF.Gelu, bias=bv[:, oc:oc + 1])
                xm = wa.tile([P, NCH, L3], F32, tag="xm")
                for c in range(NCH):
                    nc.vector.tensor_tensor(xm[:, c, :], ol[:, c, :], d0[:, c, :], OP.add)
                    nc.vector.tensor_tensor(xm[:, c, :], xm[:, c, :], d1[:, c, :], OP.add)
                dbg('xm', xm[:].rearrange("p a b -> p (a b)"), w, P, NCH * L3)

                # ---- D: diff encoder x2
                cur = xm
                for i in range(NB):
                    eb = wa.tile([P, NCH, L3], BF16, tag="eb")
                    for c in range(NCH):
                        for ly in range(NL):
                            cs = cur[:, c, ly * NF:(ly + 1) * NF]
                            nc.vector.tensor_scalar(
                                eb[:, c, ly * NF:(ly + 1) * NF], cs,
                                V.dtw[i][:, c, 1:2], None, OP.mult)
                            nc.vector.scalar_tensor_tensor(
                                eb[:, c, ly * NF + 1:(ly + 1) * NF],
                                cur[:, c, ly * NF:(ly + 1) * NF - 1],
                                V.dtw[i][:, c, 0:1],
                                eb[:, c, ly * NF + 1:(ly + 1) * NF], OP.mult, OP.add)
                            nc.vector.scalar_tensor_tensor(
                                eb[:, c, ly * NF:(ly + 1) * NF - 1],
                                cur[:, c, ly * NF + 1:(ly + 1) * NF],
                                V.dtw[i][:, c, 2:3],
                                eb[:, c, ly * NF:(ly + 1) * NF - 1], OP.mult, OP.add)
                    nxt = wa.tile([P, NCH, L3], F32, tag=f"enc{i}")
                    for oc in range(NCH):
                        ps = psum.tile([P, L3], F32, tag="pp")
                        for ic in range(NCH):
                            nc.tensor.matmul(ps[:], V.dew[i][:, ic, oc, :], eb[:, ic, :],
                                             start=(ic == 0), stop=(ic == NCH - 1))
                        nc.scalar.activation(nxt[:, oc, :], ps[:], AF.Relu, bias=V.deb[i][:, oc:oc + 1])
                    cur = nxt
                diff1 = cur
                dbg('diff1', diff1[:].rearrange("p a b -> p (a b)"), w, P, NCH * L3)

                # d1d mean accumulator (sum over layers+frames; /300 folded into weights)
                for c in range(NCH):
                    nc.vector.tensor_reduce(
                        d1sum[:, c, w:w + 1], diff1[:, c, :], mybir.AxisListType.X, OP.add)

                # ---- E: normalized features (parity-permuted) + gram + sim tiles
                sq = wa.tile([P, NCH, 600], BF16, tag="sq")
                for c in range(NCH):
                    for par in range(2):
                        for src, moff in ((ol, 0), (xm, 3)):
                            iv = src[:, c, :].rearrange("p (ly q t) -> p ly q t", ly=NL, q=50)[:, :, :, par]
                            ov = sq[:, c, :].rearrange("p (m h t) -> p m h t", m=6, h=2)[:, moff:moff + 3, par, :]
                            nc.vector.tensor_tensor(ov, iv, iv, OP.mult)
                s2a = psum.tile([1, 300], F32, tag="pp")
                s2b = psum.tile([1, 300], F32, tag="pp")
                for half, ps in ((0, s2a), (1, s2b)):
                    for c in range(NCH):
                        nc.tensor.matmul(ps[:], V.ones, sq[:, c, half * 300:(half + 1) * 300],
                                         start=(c == 0), stop=(c == NCH - 1))
                nrm = wa.tile([1, 600], F32, tag="nrm")
                nc.scalar.activation(nrm[:, 0:300], s2a[:], AF.Sqrt)
                nc.scalar.activation(nrm[:, 300:600], s2b[:], AF.Sqrt)
                nc.vector.tensor_scalar_max(nrm[:], nrm[:], 1e-8)
                rcp = wa.tile([1, 600], F32, tag="rcp")
                nc.vector.reciprocal(rcp[:], nrm[:])
                rb = wa.tile([P, 600], F32, tag="rb")
                nc.gpsimd.partition_broadcast(rb[:], rcp[:])
                # fh: per (c, m) 128 cols: [0]=0, [1:51]=odd l, [64:114]=even l
                fh = wa.tile([P, NCH, 6, P], BF16, tag="fh")
                nc.gpsimd.memset(fh[:, :, :, 0:1], 0.0)
                for c in range(NCH):
                    for par, col0 in ((1, 1), (0, 64)):
                        for src, moff in ((ol, 0), (xm, 3)):
                            iv = src[:, c, :].rearrange("p (ly q t) -> p ly q t", ly=NL, q=50)[:, :, :, par]
                            rv = rb[:].rearrange("p (m h t) -> p m h t", m=6, h=2)[:, moff:moff + 3, par, :]
                            ov = fh[:, c, moff:moff + 3, col0:col0 + 50]
                            nc.vector.tensor_tensor(ov, iv, rv, OP.mult)
                dbg('fh', fh[:].rearrange("p a b c -> p (a b c)"), w, P, NCH * 6 * P)

                gA = psum.tile([P, 384], F32, tag="pp")
                gB = psum.tile([P, 384], F32, tag="pp")
                for ps, moff in ((gA, 0), (gB, 3)):
                    for mi in range(3):
                        m = moff + mi
                        for c in range(NCH):
                            nc.tensor.matmul(
                                ps[0:114, mi * P:mi * P + 114],
                                fh[:, c, m, 0:114], fh[:, c, m, 0:114],
                                start=(c == 0), stop=(c == NCH - 1))
                for half, ps in ((0, gA), (1, gB)):
                    nc.vector.tensor_copy(simsb[b3][half][:], ps[:].rearrange("p (m j) -> p m j", m=3))

                # im2col: 54 DMAs (each a full 50x50 block; zeros come from fh col 0)
                ROW0 = {0: 0, 1: 64, 2: 1}
                for m in range(6):
                    for ky in range(3):
                        for kx in range(3):
                            r = m * 9 + ky * 3 + kx
                            st = simsb[b3][m // 3]
                            pr0 = ROW0[ky]
                            c0 = ROW0[kx]
                            nc.sync.dma_start(
                                imcol[b3][r:r + 1, :],
                                st[pr0:pr0 + 50, m % 3, c0:c0 + 50])
                dbg('imcol', imcol[b3][:], w, 54, 2500)

                # ---- F: ResNet stem + block
                # conv1 (54 -> 64, on 2500 positions); evacuate 4x:
                # h1d upper, h1e upper (plain), h1d lower (-2 col shift),
                # h1e lower (-2 row shift, clipping the first 2 rows)
                h1d, h1e = h1p[b]
                h1dv = h1d[:].rearrange("p (r c) -> p r c", r=52)
                h1ev = h1e[:].rearrange("p (r c) -> p r c", r=52)
                for chunk in range(5):
                    y0r = chunk * 10
                    ps = psum.tile([64, 500], F32, tag="pp")
                    nc.tensor.matmul(ps[:], V.c1w, imcol[b3][:, chunk * 500:(chunk + 1) * 500],
                                     start=True, stop=True)
                    pv = ps[:].rearrange("p (r c) -> p r c", c=50)
                    nc.scalar.activation(h1dv[0:64, 1 + y0r:11 + y0r, 1:51], pv,
                                         AF.Relu, bias=V.c1b)
                    nc.scalar.activation(h1ev[0:64, 1 + y0r:11 + y0r, 1:51], pv,
                                         AF.Relu, bias=V.c1b)
                    dl = h1d[64:P, y0r * 52 + 51:y0r * 52 + 51 + 520].rearrange(
                        "p (r c) -> p r c", c=52)[:, :, 0:50]
                    nc.scalar.activation(dl, pv, AF.Relu, bias=V.c1b)
                    if chunk == 0:
                        el = h1e[64:P, 1:1 + 52 * 9].rearrange(
                            "p (r c) -> p r c", c=52)[:, :, 0:50]
                        nc.scalar.activation(el, pv[:, 1:10, :], AF.Relu, bias=V.c1b)
                    else:
                        el = h1e[64:P, (y0r - 1) * 52 + 1:(y0r - 1) * 52 + 1 + 520].rearrange(
                            "p (r c) -> p r c", c=52)[:, :, 0:50]
                        nc.scalar.activation(el, pv, AF.Relu, bias=V.c1b)
                # conv2 (64 -> 256, stride 2): 4 K=128 paired matmuls + 1 K=64
                h1dq = h1d[:].rearrange("p (r q c u) -> p r q c u", r=26, q=2, c=26)
                h1eq = h1e[:].rearrange("p (r q c u) -> p r q c u", r=26, q=2, c=26)
                for oc in range(NCH):
                    for y0, ny in ((0, 13), (13, 12)):
                        ps = psum.tile([P, 512], F32, tag="pp")
                        pv = ps[:, 0:ny * 25].rearrange("p (r c) -> p r c", c=25)
                        for ky in range(3):
                            rhs = h1dq[:, y0 + ky // 2:y0 + ky // 2 + ny, ky % 2, 0:25, 0]
                            nc.tensor.matmul(pv, V.c2wp[:, ky, oc, :], rhs,
                                             start=(ky == 0), stop=False)
                        rhs = h1eq[:, y0:y0 + ny, 0, 0:25, 1]
                        nc.tensor.matmul(pv, V.c2wp[:, 3, oc, :], rhs, start=False, stop=False)
                        rhs = h1eq[0:64, y0:y0 + ny, 1, 0:25, 1]
                        nc.tensor.matmul(pv, V.c2ws[0:64, oc, :], rhs, start=False, stop=True)
                        dst = h2p[b][:, oc, :].rearrange("p (r c) -> p r c", r=27)[:, 1 + y0:1 + y0 + ny, 1:26]
                        nc.scalar.activation(dst, pv, AF.Relu, bias=V.c2b[:, oc:oc + 1])
                # b1 (3x3 s1) -> rp
                for oc in range(NCH):
                    for y0, ny in ((0, 13), (13, 12)):
                        ps = psum.tile([P, 512], F32, tag="pp")
                        pv = ps[:, 0:ny * 25].rearrange("p (r c) -> p r c", c=25)
                        k = 0
                        for ic in range(NCH):
                            hv = h2p[b][:, ic, :].rearrange("p (r c) -> p r c", r=27)
                            for ky in range(3):
                                for kx in range(3):
                                    rhs = hv[:, y0 + ky:y0 + ky + ny, kx:kx + 25]
                                    nc.tensor.matmul(pv, V.b1w[:, ic, ky * 3 + kx, oc, :], rhs,
                                                     start=(k == 0), stop=(k == 17))
                                    k += 1
                        dst = rp[b][:, oc, :].rearrange("p (r c) -> p r c", r=27)[:, 1 + y0:1 + y0 + ny, 1:26]
                        nc.scalar.activation(dst, pv, AF.Relu, bias=V.b1b[:, oc:oc + 1])
                # b2 (3x3 s1) + residual + relu; accumulate spatial sum into hsum
                accs = {0: [], 1: []}
                for oc in range(NCH):
                    for y0, ny in ((0, 13), (13, 12)):
                        ps = psum.tile([P, 512], F32, tag="pp")
                        pv = ps[:, 0:ny * 25].rearrange("p (r c) -> p r c", c=25)
                        k = 0
                        for ic in range(NCH):
                            rv = rp[b][:, ic, :].rearrange("p (r c) -> p r c", r=27)
                            for ky in range(3):
                                for kx in range(3):
                                    rhs = rv[:, y0 + ky:y0 + ky + ny, kx:kx + 25]
                                    nc.tensor.matmul(pv, V.b2w[:, ic, ky * 3 + kx, oc, :], rhs,
                                                     start=(k == 0), stop=(k == 17))
                                    k += 1
                        h2i = h2p[b][:, oc, :].rearrange("p (r c) -> p r c", r=27)[:, 1 + y0:1 + y0 + ny, 1:26]
                        ht = wa.tile([P, ny * 25], F32, tag="hfin")
                        nc.vector.scalar_tensor_tensor(
                            ht[:].rearrange("p (r c) -> p r c", c=25), pv,
                            V.b2b[:, oc:oc + 1], h2i, OP.add, OP.add)
                        scr = wa.tile([P, ny * 25], F32, tag="scr")
                        acc = pacc.tile([P, 1], F32, tag="acc")
                        nc.scalar.activation(scr[:], ht[:], AF.Relu, accum_out=acc[:])
                        accs[oc].append(acc)
                for oc in range(NCH):
                    nc.vector.tensor_tensor(hsum[:, oc, w:w + 1], accs[oc][0][:], accs[oc][1][:], OP.add)

        # ---- tail: SE + fuse over all windows
        hsb = wa.tile([P, NCH, nwin], BF16, tag="hsb")
        d1sb = wa.tile([P, NCH, nwin], BF16, tag="d1sb")
        for c in range(NCH):
            nc.scalar.activation(hsb[:, c, :], hsum[:, c, :], AF.Copy)
            nc.scalar.activation(d1sb[:, c, :], d1sum[:, c, :], AF.Copy)
        dbg('hsum', hsum[:].rearrange("p a b -> p (a b)"), 0, P, NCH * nwin)
        dbg('d1sum', d1sum[:].rearrange("p a b -> p (a b)"), 0, P, NCH * nwin)

        hcat = wa.tile([P, 4, nwin], BF16, tag="hcat")
        for wa_, wb_, src_bf, mulsrc, slot in (
                (V.se1a, V.se1b, d1sb, hsum, 0),
                (V.se2a, V.se2b, hsb, d1sum, 2)):
            ps1 = psum.tile([64, nwin], F32, tag="pp")
            for c in range(NCH):
                nc.tensor.matmul(ps1[:], wa_[:, c, :], src_bf[:, c, :],
                                 start=(c == 0), stop=(c == NCH - 1))
            s1 = wa.tile([64, nwin], BF16, tag="s1")
            nc.scalar.activation(s1[:], ps1[:], AF.Relu)
            for oc in range(NCH):
                ps2 = psum.tile([P, nwin], F32, tag="pp")
                nc.tensor.matmul(ps2[:], wb_[0:64, oc, :], s1[:], start=True, stop=True)
                attn = wa.tile([P, nwin], F32, tag="attn")
                nc.scalar.activation(attn[:], ps2[:], AF.Sigmoid)
                nc.vector.tensor_tensor(hcat[:, slot + oc, :], attn[:], mulsrc[:, oc, :], OP.mult)

        outsb = wa.tile([P, NCH, nwin], F32, tag="outsb")
        for oc in range(NCH):
            ps = psum.tile([P, nwin], F32, tag="pp")
            for kc in range(4):
                nc.tensor.matmul(ps[:], V.fusew[:, kc, oc, :], hcat[:, kc, :],
                                 start=(kc == 0), stop=(kc == 3))
            nc.scalar.activation(outsb[:, oc, :], ps[:], AF.Relu, bias=V.fuseb[:, oc:oc + 1])
        nc.sync.dma_start(outd.ap().rearrange("(c p) w -> p c w", p=P), outsb[:])

    nc.compile()
    return nc


# ----------------------------------------------------------------------------
# Host entry
# ----------------------------------------------------------------------------

def _layout_inputs(x0, x1, x2, nwin_per_core, ncores):
    """-> list of per-core xin arrays (128, NCH*nwin*300) f32."""
    A = np.stack([x0, x1, x2], axis=1)                    # (B, 3, 256, 100)
    btot = A.shape[0]
    A = A.reshape(btot, NL, NCH, P, NF)
    A = np.transpose(A, (3, 2, 0, 1, 4))                  # (128, 2, B, 3, 100)
    shards = []
    for k in range(ncores):
        s = A[:, :, k * nwin_per_core:(k + 1) * nwin_per_core]
        shards.append(np.ascontiguousarray(s.reshape(P, NCH * nwin_per_core * L3), dtype=np.float32))
    return shards


_CACHE = {}


def _get_built(nwin, repeat, debug, offsets_key, offsets):
    key = (nwin, repeat, tuple(sorted(debug)), offsets_key)
    if key not in _CACHE:
        _CACHE[key] = build_nc(nwin=nwin, repeat=repeat, debug=debug, offsets=offsets)
    return _CACHE[key]


def kernel(x0, x1, x2, params):
    x0 = np.asarray(x0, np.float32)
    x1 = np.asarray(x1, np.float32)
    x2 = np.asarray(x2, np.float32)
    wdict = prep_weights(params)
    wb, wf, offsets = pack_weights(wdict)
    offsets_key = tuple(sorted((k, v[0], v[1], v[2], v[3]) for k, v in offsets.items()))
    nc = _get_built(WPC, 1, (), offsets_key, offsets)
    shards = _layout_inputs(x0, x1, x2, WPC, NCORES)
    in_maps = [{"xin": shards[k], "wb": wb, "wf": wf} for k in range(NCORES)]
    res = run_bass_kernel_spmd(nc, in_maps, core_ids=list(range(NCORES)))
    outs = [res.results[k]["out"] for k in range(NCORES)]     # each (256, WPC)
    full = np.concatenate([o.T for o in outs], axis=0)        # (256, 256) rows=windows
    return np.ascontiguousarray(full.reshape(-1, DIM, NW), dtype=np.float32)


if __name__ == "__main__":
    # quick self-exercise with random weights (no reference comparison)
    rng = np.random.default_rng(0)
    print("kernel.py loaded OK")


# revision 18
# speedup vs baseline: 1.4135x; 1.4135x over previous
"""Trainium2 Bass kernel for nn_DiffHead_38757784879894.

Data-parallel over 8 NeuronCores: each core processes 32 of the 256
(b*nw) windows through the full network. All BN(eval) affine transforms
are folded into the adjacent linear ops on the host; matmuls run in
bf16 with fp32 PSUM accumulation; elementwise math stays fp32.

Self-contained: hardcodes all shapes; only needs numpy + the concourse
stack available in the environment.
"""
import math
from contextlib import ExitStack

import numpy as np
import ml_dtypes

import concourse.tile as tile
import concourse.bass_isa as bass_isa
from concourse import mybir, bacc
from concourse.bass_utils import run_bass_kernel_spmd

AF = mybir.ActivationFunctionType
OP = mybir.AluOpType
F32 = mybir.dt.float32
BF16 = mybir.dt.bfloat16

DIM, NL, NB, NW, NF, BTOT = 256, 3, 2, 8, 100, 256
NCORES = 8
WPC = BTOT // NCORES          # windows per core
P = 128
NCH = DIM // P                # 2 channel chunks
BNI = 1.0 / math.sqrt(1.0 + 1e-5)
L3 = NL * NF                  # 300

BF16_NP = ml_dtypes.bfloat16


# ----------------------------------------------------------------------------
# Host-side weight folding + packing
# ----------------------------------------------------------------------------

def _pack_pw(W):
    """(256,256) fp: out lhsT [kp, ic, oc, o] -> (128, 2*2*128)."""
    a = W.T.reshape(NCH, P, NCH, P)          # [ic, kp, oc, o]
    return np.transpose(a, (1, 0, 2, 3)).reshape(P, NCH * NCH * P)


def _pack_bias(b):
    """(256,) -> (128, 2) [p, chunk]."""
    return b.reshape(NCH, P).T.copy()


def prep_weights(params):
    """Fold BN/eval scales + mean-normalizers into weights; return named arrays."""
    p = {k: np.asarray(v, np.float64) for k, v in params.items()}
    out = {}

    # t_conv depthwise taps (3 layers): tw [p, c, ly, k], tbh = 0.5*bias
    tw = p['t_conv_w'].reshape(NL, DIM, 3)
    out['tw'] = ('f32', np.transpose(tw.reshape(NL, NCH, P, 3), (2, 1, 0, 3)).reshape(P, NCH * NL * 3))
    tb = p['t_conv_b'].reshape(NL, NCH, P)
    out['tbh'] = ('f32', (0.5 * np.transpose(tb, (2, 1, 0))).reshape(P, NCH * NL))

    # DiffMixer
    s_pre = p['dm_pre_g'] * BNI
    b_pre = p['dm_pre_b']
    s0 = p['dm_d0_g'] * BNI
    W0s = s0[:, None] * p['dm_d0_w']
    W0 = W0s * s_pre[None, :]
    b0 = s0 * p['dm_d0_b'] + p['dm_d0_beta'] + W0s @ b_pre
    s1 = p['dm_d1_g'] * BNI
    W1 = (s1[:, None] * p['dm_d1_w']) * s_pre[None, :]
    b1 = s1 * p['dm_d1_b'] + p['dm_d1_beta']
    out['dm0w'] = ('bf16', _pack_pw(W0))
    out['dm0b'] = ('f32', _pack_bias(b0))
    out['dm1w'] = ('bf16', _pack_pw(W1))
    out['dm1b'] = ('f32', _pack_bias(b1))

    # diff encoder blocks
    for i in range(NB):
        dt_ = p['de_dw_w'][i].reshape(DIM, 3)
        out[f'dtw{i}'] = ('f32', np.transpose(dt_.reshape(NCH, P, 3), (1, 0, 2)).reshape(P, NCH * 3))
        s = p['de_g'][i] * BNI
        Wp = s[:, None] * p['de_pw_w'][i]
        bp = s * (p['de_pw_w'][i] @ p['de_dw_b'][i] + p['de_pw_b'][i]) + p['de_beta'][i]
        out[f'de{i}w'] = ('bf16', _pack_pw(Wp))
        out[f'de{i}b'] = ('f32', _pack_bias(bp))

    # ResNet stem conv1: lhsT rows r = m*9 + ky*3 + kx, cols o (64)
    Wc1 = p['rn_c1_w'] * (p['rn_c1_g'] * BNI)[:, None, None, None]   # (64, 6, 3, 3)
    out['c1w'] = ('bf16', np.transpose(Wc1, (1, 2, 3, 0)).reshape(54, 64))
    out['c1b'] = ('f32', p['rn_c1_b'].reshape(64, 1))

    # conv2 tap-paired: 4 paired lhsT of K=128 + 1 single K=64.
    # pair i<3: rows[0:64]=tap(ky=i,kx=0), rows[64:128]=tap(ky=i,kx=2)
    # pair 3:   rows[0:64]=tap(ky=0,kx=1), rows[64:128]=tap(ky=2,kx=1)
    # single:   tap(ky=1,kx=1)
    Wc2 = p['rn_c2_w'] * (p['rn_c2_g'] * BNI)[:, None, None, None]   # (256, 64, 3, 3)
    a = np.transpose(Wc2.reshape(NCH, P, 64, 3, 3), (2, 3, 4, 0, 1))  # [ic, ky, kx, oc, o]
    pairs = np.zeros((P, 4, NCH, P))
    for ky in range(3):
        pairs[0:64, ky] = a[:, ky, 0]
        pairs[64:P, ky] = a[:, ky, 2]
    pairs[0:64, 3] = a[:, 0, 1]
    pairs[64:P, 3] = a[:, 2, 1]
    out['c2wp'] = ('bf16', pairs.reshape(P, 4 * NCH * P))
    out['c2ws'] = ('bf16', a[:, 1, 1].reshape(64, NCH * P))
    out['c2b'] = ('f32', _pack_bias(p['rn_c2_b']))

    # basic block convs: lhsT [icp, icc, t, oc, o]
    for nm, wkey, gkey, bkey in (('b1', 'rn_b1_w', 'rn_b1_g', 'rn_b1_b'),
                                 ('b2', 'rn_b2_w', 'rn_b2_g', 'rn_b2_b')):
        Wb = p[wkey] * (p[gkey] * BNI)[:, None, None, None]           # (256,256,3,3)
        a = Wb.reshape(NCH, P, NCH, P, 3, 3)                          # [oc,o,icc,icp,ky,kx]
        out[f'{nm}w'] = ('bf16', np.transpose(a, (3, 2, 4, 5, 0, 1)).reshape(P, NCH * 9 * NCH * P))
        out[f'{nm}b'] = ('f32', _pack_bias(p[bkey]))

    # SE: attn1 <- d1dsum/300 via se1_w1 ; attn2 <- hsum/625 via se2_w1
    for nm, w1k, w2k, scale in (('se1', 'se1_w1', 'se1_w2', 1.0 / 300.0),
                                ('se2', 'se2_w1', 'se2_w2', 1.0 / 625.0)):
        Wa = p[w1k] * scale                                           # (64, 256)
        a = Wa.T.reshape(NCH, P, 64)                                  # [kc, kp, o]
        out[f'{nm}a'] = ('bf16', np.transpose(a, (1, 0, 2)).reshape(P, NCH * 64))
        Wb_ = p[w2k]                                                  # (256, 64)
        a = Wb_.T.reshape(64, NCH, P)                                 # [kp, oc, o]
        out[f'{nm}b'] = ('bf16', a.reshape(64, NCH * P))

    # fuse: hcat = [attn1*hsum (scale 1/625) ; attn2*d1dsum (scale 1/300)]
    Wf = p['fuse_w'].copy()                                           # (256, 512)
    Wf[:, :DIM] *= 1.0 / 625.0
    Wf[:, DIM:] *= 1.0 / 300.0
    a = Wf.T.reshape(4, P, NCH, P)                                    # [kc, kp, oc, o]
    out['fusew'] = ('bf16', np.transpose(a, (1, 0, 2, 3)).reshape(P, 4 * NCH * P))
    out['fuseb'] = ('f32', _pack_bias(p['fuse_b']))

    out['ones'] = ('bf16', np.ones((P, 1)))
    return out


def pack_weights(wdict):
    """Pack named arrays into one (128, N) array per dtype group.

    Returns (wb_arr bf16, wf_arr f32, offsets: name -> (group, col0, rows, cols)).
    """
    offsets = {}
    cols = {'bf16': 0, 'f32': 0}
    for name, (grp, arr) in wdict.items():
        r, c = arr.shape
        offsets[name] = (grp, cols[grp], r, c)
        cols[grp] += c
    wb = np.zeros((P, cols['bf16']), dtype=BF16_NP)
    wf = np.zeros((P, cols['f32']), dtype=np.float32)
    for name, (grp, arr) in wdict.items():
        _, c0, r, c = offsets[name]
        if grp == 'bf16':
            wb[:r, c0:c0 + c] = arr.astype(BF16_NP)
        else:
            wf[:r, c0:c0 + c] = arr.astype(np.float32)
    return wb, wf, offsets


# ----------------------------------------------------------------------------
# Device program
# ----------------------------------------------------------------------------

class _Views:
    """AP views into the packed weight tiles."""

    def __init__(self, nc, wbt, wft, offsets):
        def view(name, shape):
            grp, c0, r, c = offsets[name]
            t = wbt if grp == 'bf16' else wft
            ap = t[0:r, c0:c0 + c]
            if len(shape) > 1:
                dims = ' '.join(f'a{i}' for i in range(len(shape)))
                kw = {f'a{i}': s for i, s in enumerate(shape)}
                ap = ap.rearrange(f'p ({dims}) -> p {dims}', **kw)
            return ap

        self.tw = view('tw', (NCH, NL, 3))
        self.tbh = view('tbh', (NCH, NL))
        self.dm0w = view('dm0w', (NCH, NCH, P))
        self.dm0b = view('dm0b', (NCH,))
        self.dm1w = view('dm1w', (NCH, NCH, P))
        self.dm1b = view('dm1b', (NCH,))
        self.dtw = [view(f'dtw{i}', (NCH, 3)) for i in range(NB)]
        self.dew = [view(f'de{i}w', (NCH, NCH, P)) for i in range(NB)]
        self.deb = [view(f'de{i}b', (NCH,)) for i in range(NB)]
        self.c1w = view('c1w', (64,))
        self.c1b = view('c1b', (1,))
        self.c2wp = view('c2wp', (4, NCH, P))
        self.c2ws = view('c2ws', (NCH, P))
        self.c2b = view('c2b', (NCH,))
        self.b1w = view('b1w', (NCH, 9, NCH, P))
        self.b1b = view('b1b', (NCH,))
        self.b2w = view('b2w', (NCH, 9, NCH, P))
        self.b2b = view('b2b', (NCH,))
        self.se1a = view('se1a', (NCH, 64))
        self.se1b = view('se1b', (NCH, P))
        self.se2a = view('se2a', (NCH, 64))
        self.se2b = view('se2b', (NCH, P))
        self.fusew = view('fusew', (4, NCH, P))
        self.fuseb = view('fuseb', (NCH,))
        self.ones = view('ones', (1,))


def build_nc(nwin=WPC, repeat=1, debug=(), offsets=None):
    """Build + compile the per-core program. `debug` is a set of stage names
    to dump as extra DRAM outputs (per-window slices along columns)."""
    nc = bacc.Bacc("TRN2", target_bir_lowering=False, debug=False, num_devices=1)
    nbf = max(c0 + c for g, c0, r, c in offsets.values() if g == 'bf16')
    nf32 = max(c0 + c for g, c0, r, c in offsets.values() if g == 'f32')
    xin = nc.dram_tensor("xin", [P, NCH * nwin * L3], F32, kind="ExternalInput")
    wbd = nc.dram_tensor("wb", [P, nbf], BF16, kind="ExternalInput")
    wfd = nc.dram_tensor("wf", [P, nf32], F32, kind="ExternalInput")
    outd = nc.dram_tensor("out", [DIM, nwin], F32, kind="ExternalOutput")

    dbg_t = {}

    def dbg(name, ap, w, rows, cols):
        if name not in debug:
            return
        if name not in dbg_t:
            dbg_t[name] = nc.dram_tensor(
                f"dbg_{name}", [rows, nwin * cols],
                ap.dtype, kind="ExternalOutput")
        nc.sync.dma_start(dbg_t[name].ap()[0:rows, w * cols:(w + 1) * cols], ap)

    with tile.TileContext(nc) as tc, ExitStack() as ctx:
        wpool = ctx.enter_context(tc.tile_pool(name="w", bufs=1))
        persist = ctx.enter_context(tc.tile_pool(name="persist", bufs=1))
        wa = ctx.enter_context(tc.tile_pool(name="wa", bufs=3))
        pacc = ctx.enter_context(tc.tile_pool(name="pacc", bufs=10))
        psum = ctx.enter_context(tc.tile_pool(name="psum", bufs=8, space="PSUM"))

        wbt = wpool.tile([P, nbf], BF16)
        nc.sync.dma_start(wbt[:], wbd.ap()[:])
        wft = wpool.tile([P, nf32], F32)
        nc.sync.dma_start(wft[:], wfd.ap()[:])
        V = _Views(nc, wbt, wft, offsets)

        xin_v = xin.ap().rearrange("p (c w l) -> p c w l", c=NCH, w=nwin)

        # persistent per-window-parity buffers
        hsum = persist.tile([P, NCH, nwin], F32)
        d1sum = persist.tile([P, NCH, nwin], F32)
        simsb, imcol, h1p, h2p, rp = ([], [], [], [], [])
        for b in range(2):
            simsb.append([
                persist.tile([P, 3, P], BF16, name=f"simsb{b}_0", tag=f"simsb{b}_0"),
                persist.tile([P, 3, P], BF16, name=f"simsb{b}_1", tag=f"simsb{b}_1")])
            imcol.append(persist.tile([54, 2500], BF16, name=f"imcol{b}", tag=f"imcol{b}"))
            # sim_sb[b][half]: (128, 3, 128) per m-triple; per-m 128 cols:
            # [0]=0, [1:51]=odd l, [51:64]=pad, [64:114]=even l, [114:128]=pad
            # h1d: [0:64]=h1 padded, [64:128]=same shifted +2 cols (for kx=0|2 tap pairs)
            # h1e: [0:64]=h1 padded, [64:128]=same shifted +2 rows (for ky=0|2, kx=1 pair)
            h1p.append((persist.tile([P, 52 * 52], BF16, name=f"h1d{b}", tag=f"h1d{b}"),
                        persist.tile([P, 52 * 52], BF16, name=f"h1e{b}", tag=f"h1e{b}")))
            h2p.append(persist.tile([P, NCH, 27 * 27], BF16, name=f"h2p{b}", tag=f"h2p{b}"))
            rp.append(persist.tile([P, NCH, 27 * 27], BF16, name=f"rp{b}", tag=f"rp{b}"))
            nc.gpsimd.memset(h1p[b][0][:], 0.0)
            nc.gpsimd.memset(h1p[b][1][:], 0.0)
            nc.gpsimd.memset(h2p[b][:], 0.0)
            nc.gpsimd.memset(rp[b][:], 0.0)

        for _rep in range(repeat):
            for w in range(nwin):
                b = w % 2
                b3 = w % 2
                # ---- A: load window
                x_t = wa.tile([P, NCH, L3], F32, tag="x_t")
                nc.sync.dma_start(x_t[:], xin_v[:, :, w, :])

                # ---- B: t_conv (depthwise k3 pad1) + relu + 0.5*(t+x)
                y = wa.tile([P, NCH, L3], F32, tag="y")
                th = wa.tile([P, NCH, L3], F32, tag="th")
                for c in range(NCH):
                    for ly in range(NL):
                        xs = x_t[:, c, ly * NF:(ly + 1) * NF]
                        nc.vector.tensor_scalar(
                            y[:, c, ly * NF:(ly + 1) * NF], xs,
                            V.tw[:, c, ly, 1:2], None, OP.mult)
                        nc.vector.scalar_tensor_tensor(
                            y[:, c, ly * NF + 1:(ly + 1) * NF],
                            x_t[:, c, ly * NF:(ly + 1) * NF - 1],
                            V.tw[:, c, ly, 0:1],
                            y[:, c, ly * NF + 1:(ly + 1) * NF], OP.mult, OP.add)
                        nc.vector.scalar_tensor_tensor(
                            y[:, c, ly * NF:(ly + 1) * NF - 1],
                            x_t[:, c, ly * NF + 1:(ly + 1) * NF],
                            V.tw[:, c, ly, 2:3],
                            y[:, c, ly * NF:(ly + 1) * NF - 1], OP.mult, OP.add)
                        nc.scalar.activation(
                            th[:, c, ly * NF:(ly + 1) * NF],
                            y[:, c, ly * NF:(ly + 1) * NF],
                            AF.Relu, bias=V.tbh[:, c, ly:ly + 1], scale=0.5)
                ol = wa.tile([P, NCH, L3], F32, tag="ol")
                for c in range(NCH):
                    nc.vector.scalar_tensor_tensor(
                        ol[:, c, :], x_t[:, c, :], 0.5, th[:, c, :], OP.mult, OP.add)
                dbg('ol', ol[:].rearrange("p a b -> p (a b)"), w, P, NCH * L3)

                # ---- C: DiffMixer
                olb = wa.tile([P, NCH, L3], BF16, tag="olb")
                for c in range(NCH):
                    nc.scalar.activation(olb[:, c, :], ol[:, c, :], AF.Copy)
                xd = wa.tile([P, NCH, L3], BF16, tag="xd")
                for c in range(NCH):
                    ov = ol[:, c, :].rearrange("p (ly l) -> p ly l", ly=NL)
                    xv = xd[:, c, :].rearrange("p (ly l) -> p ly l", ly=NL)
                    nc.vector.tensor_tensor(xv[:, :, 1:NF], ov[:, :, 1:NF], ov[:, :, 0:NF - 1], OP.subtract)
                    nc.vector.tensor_tensor(xv[:, :, 0:1], ov[:, :, 1:2], ov[:, :, 0:1], OP.subtract)
                d0 = wa.tile([P, NCH, L3], F32, tag="d0")
                d1 = wa.tile([P, NCH, L3], F32, tag="d1")
                for wv, bv, rhs, dst in ((V.dm0w, V.dm0b, olb, d0), (V.dm1w, V.dm1b, xd, d1)):
                    for oc in range(NCH):
                        ps = psum.tile([P, L3], F32, tag="pp")
                        for ic in range(NCH):
                            nc.tensor.matmul(ps[:], wv[:, ic, oc, :], rhs[:, ic, :],
                                             start=(ic == 0), stop=(ic == NCH - 1))
                        nc.scalar.activation(dst[:, oc, :], ps[:], AF.Gelu, bias=bv[:, oc:oc + 1])
                xm = wa.tile([P, NCH, L3], F32, tag="xm")
                for c in range(NCH):
                    nc.vector.tensor_tensor(xm[:, c, :], ol[:, c, :], d0[:, c, :], OP.add)
                    nc.vector.tensor_tensor(xm[:, c, :], xm[:, c, :], d1[:, c, :], OP.add)
                dbg('xm', xm[:].rearrange("p a b -> p (a b)"), w, P, NCH * L3)

                # ---- D: diff encoder x2
                cur = xm
                for i in range(NB):
                    eb = wa.tile([P, NCH, L3], BF16, tag="eb")
                    for c in range(NCH):
                        for ly in range(NL):
                            cs = cur[:, c, ly * NF:(ly + 1) * NF]
                            nc.vector.tensor_scalar(
                                eb[:, c, ly * NF:(ly + 1) * NF], cs,
                                V.dtw[i][:, c, 1:2], None, OP.mult)
                            nc.vector.scalar_tensor_tensor(
                                eb[:, c, ly * NF + 1:(ly + 1) * NF],
                                cur[:, c, ly * NF:(ly + 1) * NF - 1],
                                V.dtw[i][:, c, 0:1],
                                eb[:, c, ly * NF + 1:(ly + 1) * NF], OP.mult, OP.add)
                            nc.vector.scalar_tensor_tensor(
                                eb[:, c, ly * NF:(ly + 1) * NF - 1],
                                cur[:, c, ly * NF + 1:(ly + 1) * NF],
                                V.dtw[i][:, c, 2:3],
                                eb[:, c, ly * NF:(ly + 1) * NF - 1], OP.mult, OP.add)
                    nxt = wa.tile([P, NCH, L3], F32, tag=f"enc{i}")
                    for oc in range(NCH):
                        ps = psum.tile([P, L3], F32, tag="pp")
                        for ic in range(NCH):
                            nc.tensor.matmul(ps[:], V.dew[i][:, ic, oc, :], eb[:, ic, :],
                                             start=(ic == 0), stop=(ic == NCH - 1))
                        nc.scalar.activation(nxt[:, oc, :], ps[:], AF.Relu, bias=V.deb[i][:, oc:oc + 1])
                    cur = nxt
                diff1 = cur
                dbg('diff1', diff1[:].rearrange("p a b -> p (a b)"), w, P, NCH * L3)

                # d1d mean accumulator (sum over layers+frames; /300 folded into weights)
                for c in range(NCH):
                    nc.vector.tensor_reduce(
                        d1sum[:, c, w:w + 1], diff1[:, c, :], mybir.AxisListType.X, OP.add)

                # ---- E: normalized features (parity-permuted) + gram + sim tiles
                sq = wa.tile([P, NCH, 600], BF16, tag="sq")
                for c in range(NCH):
                    for par in range(2):
                        for src, moff in ((ol, 0), (xm, 3)):
                            iv = src[:, c, :].rearrange("p (ly q t) -> p ly q t", ly=NL, q=50)[:, :, :, par]
                            ov = sq[:, c, :].rearrange("p (m h t) -> p m h t", m=6, h=2)[:, moff:moff + 3, par, :]
                            nc.vector.tensor_tensor(ov, iv, iv, OP.mult)
                s2a = psum.tile([1, 300], F32, tag="pp")
                s2b = psum.tile([1, 300], F32, tag="pp")
                for half, ps in ((0, s2a), (1, s2b)):
                    for c in range(NCH):
                        nc.tensor.matmul(ps[:], V.ones, sq[:, c, half * 300:(half + 1) * 300],
                                         start=(c == 0), stop=(c == NCH - 1))
                nrm = wa.tile([1, 600], F32, tag="nrm")
                nc.scalar.activation(nrm[:, 0:300], s2a[:], AF.Sqrt)
                nc.scalar.activation(nrm[:, 300:600], s2b[:], AF.Sqrt)
                nc.vector.tensor_scalar_max(nrm[:], nrm[:], 1e-8)
                rcp = wa.tile([1, 600], F32, tag="rcp")
                nc.vector.reciprocal(rcp[:], nrm[:])
                rb = wa.tile([P, 600], F32, tag="rb")
                nc.gpsimd.partition_broadcast(rb[:], rcp[:])
                # fh: per (c, m) 128 cols: [0]=0, [1:51]=odd l, [64:114]=even l
                fh = wa.tile([P, NCH, 6, P], BF16, tag="fh")
                nc.gpsimd.memset(fh[:, :, :, 0:1], 0.0)
                for c in range(NCH):
                    for par, col0 in ((1, 1), (0, 64)):
                        for src, moff in ((ol, 0), (xm, 3)):
                            iv = src[:, c, :].rearrange("p (ly q t) -> p ly q t", ly=NL, q=50)[:, :, :, par]
                            rv = rb[:].rearrange("p (m h t) -> p m h t", m=6, h=2)[:, moff:moff + 3, par, :]
                            ov = fh[:, c, moff:moff + 3, col0:col0 + 50]
                            nc.vector.tensor_tensor(ov, iv, rv, OP.mult)
                dbg('fh', fh[:].rearrange("p a b c -> p (a b c)"), w, P, NCH * 6 * P)

                gA = psum.tile([P, 384], F32, tag="pp")
                gB = psum.tile([P, 384], F32, tag="pp")
                for ps, moff in ((gA, 0), (gB, 3)):
                    for mi in range(3):
                        m = moff + mi
                        for c in range(NCH):
                            nc.tensor.matmul(
                                ps[0:114, mi * P:mi * P + 114],
                                fh[:, c, m, 0:114], fh[:, c, m, 0:114],
                                start=(c == 0), stop=(c == NCH - 1))
                for half, ps in ((0, gA), (1, gB)):
                    nc.vector.tensor_copy(simsb[b3][half][:], ps[:].rearrange("p (m j) -> p m j", m=3))

                # im2col: 54 DMAs (each a full 50x50 block; zeros come from fh col 0)
                ROW0 = {0: 0, 1: 64, 2: 1}
                for m in range(6):
                    for ky in range(3):
                        for kx in range(3):
                            r = m * 9 + ky * 3 + kx
                            st = simsb[b3][m // 3]
                            pr0 = ROW0[ky]
                            c0 = ROW0[kx]
                            nc.sync.dma_start(
                                imcol[b3][r:r + 1, :],
                                st[pr0:pr0 + 50, m % 3, c0:c0 + 50])
                dbg('imcol', imcol[b3][:], w, 54, 2500)

                # ---- F: ResNet stem + block
                # conv1 (54 -> 64, on 2500 positions); evacuate 4x:
                # h1d upper, h1e upper (plain), h1d lower (-2 col shift),
                # h1e lower (-2 row shift, clipping the first 2 rows)
                h1d, h1e = h1p[b]
                h1dv = h1d[:].rearrange("p (r c) -> p r c", r=52)
                h1ev = h1e[:].rearrange("p (r c) -> p r c", r=52)
                for chunk in range(5):
                    y0r = chunk * 10
                    ps = psum.tile([64, 500], F32, tag="pp")
                    nc.tensor.matmul(ps[:], V.c1w, imcol[b3][:, chunk * 500:(chunk + 1) * 500],
                                     start=True, stop=True)
                    pv = ps[:].rearrange("p (r c) -> p r c", c=50)
                    nc.scalar.activation(h1dv[0:64, 1 + y0r:11 + y0r, 1:51], pv,
                                         AF.Relu, bias=V.c1b)
                    nc.scalar.activation(h1ev[0:64, 1 + y0r:11 + y0r, 1:51], pv,
                                         AF.Relu, bias=V.c1b)
                    dl = h1d[64:P, y0r * 52 + 51:y0r * 52 + 51 + 520].rearrange(
                        "p (r c) -> p r c", c=52)[:, :, 0:50]
                    nc.scalar.activation(dl, pv, AF.Relu, bias=V.c1b)
                    if chunk == 0:
                        el = h1e[64:P, 1:1 + 52 * 9].rearrange(
                            "p (r c) -> p r c", c=52)[:, :, 0:50]
                        nc.scalar.activation(el, pv[:, 1:10, :], AF.Relu, bias=V.c1b)
                    else:
                        el = h1e[64:P, (y0r - 1) * 52 + 1:(y0r - 1) * 52 + 1 + 520].rearrange(
                            "p (r c) -> p r c", c=52)[:, :, 0:50]
                        nc.scalar.activation(el, pv, AF.Relu, bias=V.c1b)
                # conv2 (64 -> 256, stride 2): 4 K=128 paired matmuls + 1 K=64
                h1dq = h1d[:].rearrange("p (r q c u) -> p r q c u", r=26, q=2, c=26)
                h1eq = h1e[:].rearrange("p (r q c u) -> p r q c u", r=26, q=2, c=26)
                for oc in range(NCH):
                    for y0, ny in ((0, 13), (13, 12)):
                        ps = psum.tile([P, 512], F32, tag="pp")
                        pv = ps[:, 0:ny * 25].rearrange("p (r c) -> p r c", c=25)
                        for ky in range(3):
                            rhs = h1dq[:, y0 + ky // 2:y0 + ky // 2 + ny, ky % 2, 0:25, 0]
                            nc.tensor.matmul(pv, V.c2wp[:, ky, oc, :], rhs,
                                             start=(ky == 0), stop=False)
                        rhs = h1eq[:, y0:y0 + ny, 0, 0:25, 1]
                        nc.tensor.matmul(pv, V.c2wp[:, 3, oc, :], rhs, start=False, stop=False)
                        rhs = h1eq[0:64, y0:y0 + ny, 1, 0:25, 1]
                        nc.tensor.matmul(pv, V.c2ws[0:64, oc, :], rhs, start=False, stop=True)
                        dst = h2p[b][:, oc, :].rearrange("p (r c) -> p r c", r=27)[:, 1 + y0:1 + y0 + ny, 1:26]
                        nc.scalar.activation(dst, pv, AF.Relu, bias=V.c2b[:, oc:oc + 1])
                # b1 (3x3 s1) -> rp
                for oc in range(NCH):
                    for y0, ny in ((0, 13), (13, 12)):
                        ps = psum.tile([P, 512], F32, tag="pp")
                        pv = ps[:, 0:ny * 25].rearrange("p (r c) -> p r c", c=25)
                        k = 0
                        for ic in range(NCH):
                            hv = h2p[b][:, ic, :].rearrange("p (r c) -> p r c", r=27)
                            for ky in range(3):
                                for kx in range(3):
                                    rhs = hv[:, y0 + ky:y0 + ky + ny, kx:kx + 25]
                                    nc.tensor.matmul(pv, V.b1w[:, ic, ky * 3 + kx, oc, :], rhs,
                                                     start=(k == 0), stop=(k == 17))
                                    k += 1
                        dst = rp[b][:, oc, :].rearrange("p (r c) -> p r c", r=27)[:, 1 + y0:1 + y0 + ny, 1:26]
                        nc.scalar.activation(dst, pv, AF.Relu, bias=V.b1b[:, oc:oc + 1])
                # b2 (3x3 s1) + residual + relu; accumulate spatial sum into hsum
                accs = {0: [], 1: []}
                for oc in range(NCH):
                    for y0, ny in ((0, 13), (13, 12)):
                        ps = psum.tile([P, 512], F32, tag="pp")
                        pv = ps[:, 0:ny * 25].rearrange("p (r c) -> p r c", c=25)
                        k = 0
                        for ic in range(NCH):
                            rv = rp[b][:, ic, :].rearrange("p (r c) -> p r c", r=27)
                            for ky in range(3):
                                for kx in range(3):
                                    rhs = rv[:, y0 + ky:y0 + ky + ny, kx:kx + 25]
                                    nc.tensor.matmul(pv, V.b2w[:, ic, ky * 3 + kx, oc, :], rhs,
                                                     start=(k == 0), stop=(k == 17))
                                    k += 1
                        h2i = h2p[b][:, oc, :].rearrange("p (r c) -> p r c", r=27)[:, 1 + y0:1 + y0 + ny, 1:26]
                        ht = wa.tile([P, ny * 25], F32, tag="hfin")
                        nc.vector.scalar_tensor_tensor(
                            ht[:].rearrange("p (r c) -> p r c", c=25), pv,
                            V.b2b[:, oc:oc + 1], h2i, OP.add, OP.add)
                        scr = wa.tile([P, ny * 25], F32, tag="scr")
                        acc = pacc.tile([P, 1], F32, tag="acc")
                        nc.scalar.activation(scr[:], ht[:], AF.Relu, accum_out=acc[:])
                        accs[oc].append(acc)
                for oc in range(NCH):
                    nc.vector.tensor_tensor(hsum[:, oc, w:w + 1], accs[oc][0][:], accs[oc][1][:], OP.add)

        # ---- tail: SE + fuse over all windows
        hsb = wa.tile([P, NCH, nwin], BF16, tag="hsb")
        d1sb = wa.tile([P, NCH, nwin], BF16, tag="d1sb")
        for c in range(NCH):
            nc.scalar.activation(hsb[:, c, :], hsum[:, c, :], AF.Copy)
            nc.scalar.activation(d1sb[:, c, :], d1sum[:, c, :], AF.Copy)
        dbg('hsum', hsum[:].rearrange("p a b -> p (a b)"), 0, P, NCH * nwin)
        dbg('d1sum', d1sum[:].rearrange("p a b -> p (a b)"), 0, P, NCH * nwin)

        hcat = wa.tile([P, 4, nwin], BF16, tag="hcat")
        for wa_, wb_, src_bf, mulsrc, slot in (
                (V.se1a, V.se1b, d1sb, hsum, 0),
                (V.se2a, V.se2b, hsb, d1sum, 2)):
            ps1 = psum.tile([64, nwin], F32, tag="pp")
            for c in range(NCH):
                nc.tensor.matmul(ps1[:], wa_[:, c, :], src_bf[:, c, :],
                                 start=(c == 0), stop=(c == NCH - 1))
            s1 = wa.tile([64, nwin], BF16, tag="s1")
            nc.scalar.activation(s1[:], ps1[:], AF.Relu)
            for oc in range(NCH):
                ps2 = psum.tile([P, nwin], F32, tag="pp")
                nc.tensor.matmul(ps2[:], wb_[0:64, oc, :], s1[:], start=True, stop=True)
                attn = wa.tile([P, nwin], F32, tag="attn")
                nc.scalar.activation(attn[:], ps2[:], AF.Sigmoid)
                nc.vector.tensor_tensor(hcat[:, slot + oc, :], attn[:], mulsrc[:, oc, :], OP.mult)

        outsb = wa.tile([P, NCH, nwin], F32, tag="outsb")
        for oc in range(NCH):
            ps = psum.tile([P, nwin], F32, tag="pp")
            for kc in range(4):
                nc.tensor.matmul(ps[:], V.fusew[:, kc, oc, :], hcat[:, kc, :],
                                 start=(kc == 0), stop=(kc == 3))
            nc.scalar.activation(outsb[:, oc, :], ps[:], AF.Relu, bias=V.fuseb[:, oc:oc + 1])
        nc.sync.dma_start(outd.ap().rearrange("(c p) w -> p c w", p=P), outsb[:])

    nc.compile()
    return nc


# ----------------------------------------------------------------------------
# Host entry
# ----------------------------------------------------------------------------

def _layout_inputs(x0, x1, x2, nwin_per_core, ncores):
    """-> list of per-core xin arrays (128, NCH*nwin*300) f32."""
    A = np.stack([x0, x1, x2], axis=1)                    # (B, 3, 256, 100)
    btot = A.shape[0]
    A = A.reshape(btot, NL, NCH, P, NF)
    A = np.transpose(A, (3, 2, 0, 1, 4))                  # (128, 2, B, 3, 100)
    shards = []
    for k in range(ncores):
        s = A[:, :, k * nwin_per_core:(k + 1) * nwin_per_core]
        shards.append(np.ascontiguousarray(s.reshape(P, NCH * nwin_per_core * L3), dtype=np.float32))
    return shards


_CACHE = {}


def _get_built(nwin, repeat, debug, offsets_key, offsets):
    key = (nwin, repeat, tuple(sorted(debug)), offsets_key)
    if key not in _CACHE:
        _CACHE[key] = build_nc(nwin=nwin, repeat=repeat, debug=debug, offsets=offsets)
    return _CACHE[key]


def kernel(x0, x1, x2, params):
    x0 = np.asarray(x0, np.float32)
    x1 = np.asarray(x1, np.float32)
    x2 = np.asarray(x2, np.float32)
    wdict = prep_weights(params)
    wb, wf, offsets = pack_weights(wdict)
    offsets_key = tuple(sorted((k, v[0], v[1], v[2], v[3]) for k, v in offsets.items()))
    nc = _get_built(WPC, 1, (), offsets_key, offsets)
    shards = _layout_inputs(x0, x1, x2, WPC, NCORES)
    in_maps = [{"xin": shards[k], "wb": wb, "wf": wf} for k in range(NCORES)]
    res = run_bass_kernel_spmd(nc, in_maps, core_ids=list(range(NCORES)))
    outs = [res.results[k]["out"] for k in range(NCORES)]     # each (256, WPC)
    full = np.concatenate([o.T for o in outs], axis=0)        # (256, 256) rows=windows
    return np.ascontiguousarray(full.reshape(-1, DIM, NW), dtype=np.float32)


if __name__ == "__main__":
    # quick self-exercise with random weights (no reference comparison)
    rng = np.random.default_rng(0)
    print("kernel.py loaded OK")
